# revision 8
# baseline (speedup 1.0000x reference)
"""Trainium2 Bass kernel for nn_Attention_63127429317226.

out[d] = sum_t softmax_d(c_d * q_t)[t, d] * q_t[t, d],  c = W * r_star
  T = 32768, D = 1024.  (The scalar bias b is softmax-invariant and drops out.)

Reformulation: with beta[t,d] = c_d * q[t,d] (host-precomputed input prep,
fp16 — invertible elementwise scaling of q),

  out[d] * c_d = sum_t (1/s[t]) * beta[t,d] * e^beta[t,d],   s[t] = sum_d e^beta

Per [128, 1024] tile on device (T sharded 8 ways -> 32 tiles/core):
  e   = exp(beta)      ACT, fused over 4-tile groups (N up to 4096)
  s   = rowsum(e)      split: 1 tile/group via the ACT accumulator, the rest
                       via a DVE pairwise-add tree (TT-adds run in the 2x
                       fp16 perf mode where the 1x-only reduce path cannot)
                       finished by one small tensor_reduce
  hb  = beta * e       DVE tensor_tensor (2x fp16), fused over the group
  r   = 1/s            DVE reciprocal, batched per group -> fp16
  PSUM[1, 1024] += r^T @ hb   PE: 2 matmuls (N=512) per tile; the softmax
                       normalization rides in the stationary operand, the
                       t-reduction in the PSUM accumulation.
Epilogue: copy the [1, 1024] f32 PSUM row to SBUF, DMA out; host sums the 8
cores' partials in f64, divides by c_d, returns f32.
"""

import os
import sys
from contextlib import ExitStack

import numpy as np

for _p in ("/opt/trn_rl_repo", "/root/.axon_site/_ro/trn_rl_repo"):
    if os.path.isdir(_p) and _p not in sys.path:
        sys.path.insert(0, _p)

import concourse.bacc as bacc
import concourse.tile as tile
from concourse import mybir
from concourse.bass_utils import run_bass_kernel_spmd

D = 1024
T = 32768
N_CORES = 8
P = 128

F32 = mybir.dt.float32
FP16 = mybir.dt.float16

# (group_size, n_leading_tiles_whose_s_comes_from_the_ACT_accumulator)
GROUPS = [(1, 1), (1, 1), (2, 0)] + [(4, 1)] * 7


def build_nc(t_shard: int):
    """Build the single-core Bass program for a T-shard of `t_shard` rows."""
    assert t_shard % P == 0
    n_tiles = t_shard // P
    assert sum(g for g, _ in GROUPS) == n_tiles

    nc = bacc.Bacc(None)
    beta = nc.dram_tensor("beta", [t_shard, D], FP16, kind="ExternalInput")
    out = nc.dram_tensor("out", [1, D], F32, kind="ExternalOutput")

    import types as _types

    from concourse.vector_clock import ScopedClock as _ScopedClock

    def _minimal_drain(self, tick_clock, wait_clock):
        # Slim kernel exit: keep the completion-join drain but skip the two
        # all-engine barriers + sem-clear instructions — the Bass preamble
        # re-clears the sem range at the start of every execution.
        drain_inst = self.nc.sync.drain()
        wait_clock.add_sem_waits(
            drain_inst.ins, _ScopedClock({None: tick_clock.global_clock})
        )
        popped = self.nc._tile_sem_poison_stack.pop()
        assert popped is self._sem_poison

    Exp = mybir.ActivationFunctionType.Exp
    ADD = mybir.AluOpType.add

    with tile.TileContext(nc) as tc, ExitStack() as ctx:
        if os.environ.get("KERNEL_FASTEXIT", "1") == "1":
            tc._drain_and_barrier = _types.MethodType(_minimal_drain, tc)
        singles = ctx.enter_context(tc.tile_pool(name="singles", bufs=1))
        bpool = ctx.enter_context(tc.tile_pool(name="bpool", bufs=5))
        epool = ctx.enter_context(tc.tile_pool(name="epool", bufs=4))
        hpool = ctx.enter_context(tc.tile_pool(name="hpool", bufs=4))
        t1pool = ctx.enter_context(tc.tile_pool(name="t1pool", bufs=2))
        t2pool = ctx.enter_context(tc.tile_pool(name="t2pool", bufs=2))
        t3pool = ctx.enter_context(tc.tile_pool(name="t3pool", bufs=2))
        t4pool = ctx.enter_context(tc.tile_pool(name="t4pool", bufs=2))
        spool = ctx.enter_context(tc.tile_pool(name="spool", bufs=4))
        rpool = ctx.enter_context(tc.tile_pool(name="rpool", bufs=4))
        psum = ctx.enter_context(tc.tile_pool(name="psum", bufs=1, space="PSUM"))

        acc = psum.tile([1, 2, 512], F32)

        i0 = 0
        for g, n_acc in GROUPS:
            tiles = list(range(i0, i0 + g))

            bg = bpool.tile([P, 4, D], FP16, name="bg")
            nc.sync.dma_start(
                out=bg[:, :g, :],
                in_=beta[i0 * P : (i0 + g) * P, :].rearrange(
                    "(a p) d -> p a d", p=P
                ),
            )
            # s / r live in per-group pool tiles: a shared static tile would
            # make each group's recip wait (WAR) on the previous group's
            # matmuls, serializing the whole pipeline.
            sg = spool.tile([P, 4], F32, name="sg")
            rg = rpool.tile([P, 4], FP16, name="rg")
            eg = epool.tile([P, 4, D], FP16, name="eg")
            # exp; the leading n_acc tiles get their row-sum from the ACT
            # accumulator (solo ACTIVATE each), the rest share one ACTIVATE.
            for a in range(n_acc):
                nc.scalar.activation(
                    eg[:, a : a + 1, :],
                    bg[:, a : a + 1, :],
                    Exp,
                    accum_out=sg[:, a : a + 1],
                )
            if n_acc < g:
                nc.scalar.activation(eg[:, n_acc:g, :], bg[:, n_acc:g, :], Exp)

            hbg = hpool.tile([P, 4, D], FP16, name="hbg")
            nc.vector.tensor_mul(hbg[:, :g, :], bg[:, :g, :], eg[:, :g, :])

            # s for the remaining tiles: 4-level pairwise add tree (2x mode)
            # + one small 1x reduce into f32.
            if n_acc < g:
                lo, hi = n_acc, g
                t1 = t1pool.tile([P, 4, 512], FP16, name="t1")
                nc.vector.tensor_add(
                    t1[:, lo:hi, :], eg[:, lo:hi, 0:512], eg[:, lo:hi, 512:1024]
                )
                t2 = t2pool.tile([P, 4, 256], FP16, name="t2")
                nc.vector.tensor_add(
                    t2[:, lo:hi, :], t1[:, lo:hi, 0:256], t1[:, lo:hi, 256:512]
                )
                t3 = t3pool.tile([P, 4, 128], FP16, name="t3")
                nc.vector.tensor_add(
                    t3[:, lo:hi, :], t2[:, lo:hi, 0:128], t2[:, lo:hi, 128:256]
                )
                t4 = t4pool.tile([P, 4, 64], FP16, name="t4")
                nc.vector.tensor_add(
                    t4[:, lo:hi, :], t3[:, lo:hi, 0:64], t3[:, lo:hi, 64:128]
                )
                nc.vector.tensor_reduce(
                    sg[:, lo:hi],
                    t4[:, lo:hi, :],
                    axis=mybir.AxisListType.X,
                    op=ADD,
                )

            with nc.allow_low_precision("fp16 r: 2.4e-4 rel, tol is 2e-2"):
                nc.vector.reciprocal(rg[:, :g], sg[:, :g])

            for a, i in enumerate(tiles):
                for h in range(2):
                    nc.tensor.matmul(
                        acc[:, h, :],
                        rg[:, a : a + 1],
                        hbg[:, a, h * 512 : (h + 1) * 512],
                        start=(i == 0),
                        stop=(i == n_tiles - 1),
                    )
            i0 += g

        fin = singles.tile([1, 2, 512], F32)
        nc.scalar.copy(fin[:, :, :], acc[:, :, :])
        nc.sync.dma_start(
            out=out[0:1, :].rearrange("p (a j) -> p a j", a=2), in_=fin[:, :, :]
        )

    nc.compile()
    return nc


_NC_CACHE: dict = {}


def _get_nc(t_shard: int):
    if t_shard not in _NC_CACHE:
        _NC_CACHE[t_shard] = build_nc(t_shard)
    return _NC_CACHE[t_shard]


def _make_beta(q_t: np.ndarray, w: np.ndarray, r_star: np.ndarray) -> np.ndarray:
    c = (w.astype(np.float64) * r_star.astype(np.float64)).astype(np.float32)
    return (q_t * c[None, :]).astype(np.float16)


def kernel(**inputs) -> np.ndarray:
    q_t = np.ascontiguousarray(np.asarray(inputs["q_t"], dtype=np.float32))
    r_star = np.asarray(inputs["r_star"], dtype=np.float32)
    w = np.asarray(inputs["W"], dtype=np.float32)
    # inputs["b"] is a scalar bias added uniformly before a softmax over d:
    # softmax(x + c) == softmax(x), so it cannot affect the output.

    t_total = q_t.shape[0]
    t_shard = t_total // N_CORES
    nc = _get_nc(t_shard)

    beta = _make_beta(q_t, w, r_star)
    shards = beta.reshape(N_CORES, t_shard, D)
    in_maps = [{"beta": shards[c]} for c in range(N_CORES)]
    res = run_bass_kernel_spmd(nc, in_maps, core_ids=list(range(N_CORES)))
    parts = np.stack(
        [res.results[c]["out"].reshape(D) for c in range(N_CORES)]
    )  # [8, 1024]
    total = parts.astype(np.float64).sum(axis=0)  # [1024]
    c = w.astype(np.float64) * r_star.astype(np.float64)
    return (total / c).astype(np.float32)


# revision 11
# speedup vs baseline: 1.0492x; 1.0492x over previous
"""Trainium2 Bass kernel for nn_Attention_63127429317226.

out[d] = sum_t softmax_d(c_d * q_t)[t, d] * q_t[t, d],  c = W * r_star
  T = 32768, D = 1024.  (The scalar bias b is softmax-invariant and drops out.)

Reformulation: with beta[t,d] = c_d * q[t,d] (host-precomputed input prep,
fp16 — invertible elementwise scaling of q),

  out[d] * c_d = sum_t (1/s[t]) * beta[t,d] * e^beta[t,d],   s[t] = sum_d e^beta

Per [128, 1024] tile on device (T sharded 8 ways -> 32 tiles/core):
  e   = exp(beta)      ACT, fused over 4-tile groups (N up to 4096)
  s   = rowsum(e)      split: 1 tile/group via the ACT accumulator, the rest
                       via a DVE pairwise-add tree (TT-adds run in the 2x
                       fp16 perf mode where the 1x-only reduce path cannot)
                       finished by one small tensor_reduce
  hb  = beta * e       DVE tensor_tensor (2x fp16), fused over the group
  r   = 1/s            DVE reciprocal, batched per group -> fp16
  PSUM[1, 1024] += r^T @ hb   PE: 2 matmuls (N=512) per tile; the softmax
                       normalization rides in the stationary operand, the
                       t-reduction in the PSUM accumulation.
Epilogue: copy the [1, 1024] f32 PSUM row to SBUF, DMA out; host sums the 8
cores' partials in f64, divides by c_d, returns f32.
"""

import os
import sys
from contextlib import ExitStack

import numpy as np

for _p in ("/opt/trn_rl_repo", "/root/.axon_site/_ro/trn_rl_repo"):
    if os.path.isdir(_p) and _p not in sys.path:
        sys.path.insert(0, _p)

import concourse.bacc as bacc
import concourse.tile as tile
from concourse import mybir
from concourse.bass_utils import run_bass_kernel_spmd

D = 1024
T = 32768
N_CORES = 8
P = 128

F32 = mybir.dt.float32
FP16 = mybir.dt.float16

# (group_size, n_leading_tiles_whose_s_comes_from_the_ACT_accumulator)
# Small head groups fill the pipeline fast; the tiny tail group keeps the
# post-last-ACTIVATE drain chain (DVE -> PE -> copy -> DMA) short.
GROUPS = [(1, 1), (2, 0), (4, 1)] * 1 + [(4, 1)] * 6 + [(1, 1)]


def build_nc(t_shard: int):
    """Build the single-core Bass program for a T-shard of `t_shard` rows."""
    assert t_shard % P == 0
    n_tiles = t_shard // P
    assert sum(g for g, _ in GROUPS) == n_tiles

    nc = bacc.Bacc(None)
    beta = nc.dram_tensor("beta", [t_shard, D], FP16, kind="ExternalInput")
    out = nc.dram_tensor("out", [1, D], F32, kind="ExternalOutput")

    import types as _types

    from concourse.vector_clock import ScopedClock as _ScopedClock

    def _minimal_drain(self, tick_clock, wait_clock):
        # Slim kernel exit: keep the completion-join drain but skip the two
        # all-engine barriers + sem-clear instructions — the Bass preamble
        # re-clears the sem range at the start of every execution.
        drain_inst = self.nc.sync.drain()
        wait_clock.add_sem_waits(
            drain_inst.ins, _ScopedClock({None: tick_clock.global_clock})
        )
        popped = self.nc._tile_sem_poison_stack.pop()
        assert popped is self._sem_poison

    Exp = mybir.ActivationFunctionType.Exp
    ADD = mybir.AluOpType.add

    with tile.TileContext(nc) as tc, ExitStack() as ctx:
        if os.environ.get("KERNEL_FASTEXIT", "1") == "1":
            tc._drain_and_barrier = _types.MethodType(_minimal_drain, tc)
        singles = ctx.enter_context(tc.tile_pool(name="singles", bufs=1))
        bpool = ctx.enter_context(tc.tile_pool(name="bpool", bufs=5))
        epool = ctx.enter_context(tc.tile_pool(name="epool", bufs=4))
        hpool = ctx.enter_context(tc.tile_pool(name="hpool", bufs=4))
        t1pool = ctx.enter_context(tc.tile_pool(name="t1pool", bufs=2))
        spool = ctx.enter_context(tc.tile_pool(name="spool", bufs=4))
        rpool = ctx.enter_context(tc.tile_pool(name="rpool", bufs=4))
        psum = ctx.enter_context(tc.tile_pool(name="psum", bufs=1, space="PSUM"))

        acc = psum.tile([1, 2, 512], F32)

        i0 = 0
        for g, n_acc in GROUPS:
            tiles = list(range(i0, i0 + g))

            bg = bpool.tile([P, 4, D], FP16, name="bg")
            nc.sync.dma_start(
                out=bg[:, :g, :],
                in_=beta[i0 * P : (i0 + g) * P, :].rearrange(
                    "(a p) d -> p a d", p=P
                ),
            )
            # s / r live in per-group pool tiles: a shared static tile would
            # make each group's recip wait (WAR) on the previous group's
            # matmuls, serializing the whole pipeline.
            sg = spool.tile([P, 4], F32, name="sg")
            rg = rpool.tile([P, 4], FP16, name="rg")
            eg = epool.tile([P, 4, D], FP16, name="eg")
            # exp; the leading n_acc tiles get their row-sum from the ACT
            # accumulator (solo ACTIVATE each), the rest share one ACTIVATE.
            for a in range(n_acc):
                nc.scalar.activation(
                    eg[:, a : a + 1, :],
                    bg[:, a : a + 1, :],
                    Exp,
                    accum_out=sg[:, a : a + 1],
                )
            if n_acc < g:
                nc.scalar.activation(eg[:, n_acc:g, :], bg[:, n_acc:g, :], Exp)

            hbg = hpool.tile([P, 4, D], FP16, name="hbg")
            nc.vector.tensor_mul(hbg[:, :g, :], bg[:, :g, :], eg[:, :g, :])

            # s for the remaining tiles: one scalar_tensor_tensor per tile
            # computes the 512-wide pairsum (e_lo + e_hi) AND its row-sum via
            # the op's accumulator output -> s in a single instruction.
            if n_acc < g:
                t1 = t1pool.tile([P, 4, 512], FP16, name="t1")
                for a in range(n_acc, g):
                    nc.vector.scalar_tensor_tensor(
                        t1[:, a, :],
                        eg[:, a, 0:512],
                        0.0,
                        eg[:, a, 512:1024],
                        ADD,
                        ADD,
                        accum_out=sg[:, a : a + 1],
                    )

            with nc.allow_low_precision("fp16 r: 2.4e-4 rel, tol is 2e-2"):
                nc.vector.reciprocal(rg[:, :g], sg[:, :g])

            for a, i in enumerate(tiles):
                for h in range(2):
                    nc.tensor.matmul(
                        acc[:, h, :],
                        rg[:, a : a + 1],
                        hbg[:, a, h * 512 : (h + 1) * 512],
                        start=(i == 0),
                        stop=(i == n_tiles - 1),
                    )
            i0 += g

        fin = singles.tile([1, 2, 512], F32)
        nc.scalar.copy(fin[:, :, :], acc[:, :, :])
        nc.sync.dma_start(
            out=out[0:1, :].rearrange("p (a j) -> p a j", a=2), in_=fin[:, :, :]
        )

    nc.compile()
    return nc


_NC_CACHE: dict = {}


def _get_nc(t_shard: int):
    if t_shard not in _NC_CACHE:
        _NC_CACHE[t_shard] = build_nc(t_shard)
    return _NC_CACHE[t_shard]


def _make_beta(q_t: np.ndarray, w: np.ndarray, r_star: np.ndarray) -> np.ndarray:
    c = (w.astype(np.float64) * r_star.astype(np.float64)).astype(np.float32)
    return (q_t * c[None, :]).astype(np.float16)


def kernel(**inputs) -> np.ndarray:
    q_t = np.ascontiguousarray(np.asarray(inputs["q_t"], dtype=np.float32))
    r_star = np.asarray(inputs["r_star"], dtype=np.float32)
    w = np.asarray(inputs["W"], dtype=np.float32)
    # inputs["b"] is a scalar bias added uniformly before a softmax over d:
    # softmax(x + c) == softmax(x), so it cannot affect the output.

    t_total = q_t.shape[0]
    t_shard = t_total // N_CORES
    nc = _get_nc(t_shard)

    beta = _make_beta(q_t, w, r_star)
    shards = beta.reshape(N_CORES, t_shard, D)
    in_maps = [{"beta": shards[c]} for c in range(N_CORES)]
    res = run_bass_kernel_spmd(nc, in_maps, core_ids=list(range(N_CORES)))
    parts = np.stack(
        [res.results[c]["out"].reshape(D) for c in range(N_CORES)]
    )  # [8, 1024]
    total = parts.astype(np.float64).sum(axis=0)  # [1024]
    c = w.astype(np.float64) * r_star.astype(np.float64)
    return (total / c).astype(np.float32)
